# revision 4
# baseline (speedup 1.0000x reference)
"""CharacterAwareEncoder kernel for Trainium2 (8 NeuronCores, data-parallel).

reference:
    word_embeds  = word_emb_table[word_ids]                  # [B, S, 412] gather
    char_features = sin(freqs * word_ids), 0 where id == 0   # [B, S, 100]
    out = concat([word_embeds, char_features], -1)           # [B, S, 512]

Sharding: word_ids [16, 2048] flattened to 32768 tokens, 4096 per core;
embedding table replicated. Per core: 32 tiles of 128 tokens; each tile's
rows are gathered straight into the first 412 columns of a [128, 512]
output slice via indirect DMA, the sinusoidal features are computed with
a Cody-Waite range reduction + ACT-engine Sin into the last 100 columns,
and the fused [128, 512] rows are stored contiguously.

sin accuracy: x = freq*tok <= 3168 rad.  k = int(x / 2pi) (either trunc or
round-to-nearest hardware cast works), r = ((x - k*c1) - k*c2) - k*c3 with a
3-term Cody-Waite split of 2pi, then a +-2pi range wrap (fixes any off-by-one
k) and a clamp to +-PI_SAFE so the ACT Sin table (valid on [-pi, pi]) never
sees an out-of-domain value.  Max abs error vs float64 sin ~4e-7.
"""

import numpy as np

import concourse.bacc as bacc
import concourse.bass as bass
import concourse.mybir as mybir
import concourse.tile as tile
from concourse.bass_utils import run_bass_kernel_spmd

B, S = 16, 2048
V, D, H = 32000, 412, 100
OUT_D = 512
N_CORES = 8
P = 128
T_CORE = B * S // N_CORES          # 4096 tokens per core
N_TILES = T_CORE // P              # 32 tiles of 128 tokens
CHUNK_TILES = 8                    # tiles per double-buffered SBUF chunk
N_CHUNKS = N_TILES // CHUNK_TILES  # 4
W = CHUNK_TILES * H                # sin-pipeline width per chunk

_f32 = mybir.dt.float32
_i32 = mybir.dt.int32

# Cody-Waite split of 2*pi: c1/c2 keep 12 mantissa bits so k*c1, k*c2 are
# exact for k <= 505; c3 absorbs the rest (residual ~7e-15).
_TWO_PI = 2.0 * np.pi
def _split_high(v):
    f = np.float32(v)
    return (f.view(np.uint32) & np.uint32(0xFFFFF000)).view(np.float32)
C1 = float(_split_high(_TWO_PI))
C2 = float(_split_high(_TWO_PI - C1))
C3 = float(np.float32(_TWO_PI - C1 - C2))
INV2PI = float(np.float32(1.0 / _TWO_PI))
PI_F32 = float(np.float32(np.pi))
TWO_PI_F32 = float(np.float32(_TWO_PI))
PI_SAFE = float(np.nextafter(np.float32(np.pi), np.float32(0)))  # < float64 pi

_NC = None


def _build_nc():
    # Bacc (not plain Bass): its compile() pass splits multi-semaphore waits
    # into InstEventSemaphore chains — TRN2 compute instructions encode at
    # most one sync wait, and walrus refuses to legalize this itself.
    nc = bacc.Bacc("TRN2", target_bir_lowering=False)
    ids_t = nc.dram_tensor("ids", [P, N_TILES], _i32, kind="ExternalInput")
    freqs_t = nc.dram_tensor("freqs", [P, W], _f32, kind="ExternalInput")
    table_t = nc.dram_tensor("table", [V, D], _f32, kind="ExternalInput")
    out_t = nc.dram_tensor("out", [T_CORE, OUT_D], _f32, kind="ExternalOutput")

    with tile.TileContext(nc) as tc:
        with (
            tc.tile_pool(name="const", bufs=1) as cpool,
            tc.tile_pool(name="chunks", bufs=3) as chpool,
            tc.tile_pool(name="work", bufs=2) as wpool,
        ):
            ids_sb = cpool.tile([P, N_TILES], _i32)
            nc.sync.dma_start(out=ids_sb[:], in_=ids_t[:])
            freqs_sb = cpool.tile([P, W], _f32)
            nc.sync.dma_start(out=freqs_sb[:], in_=freqs_t[:])
            tokf = cpool.tile([P, N_TILES], _f32)
            nc.vector.tensor_copy(out=tokf[:], in_=ids_sb[:])  # exact int->f32

            for g in range(N_CHUNKS):
                ch = chpool.tile([P, CHUNK_TILES, OUT_D], _f32, tag="ch")
                for j in range(CHUNK_TILES):
                    t = g * CHUNK_TILES + j
                    nc.gpsimd.indirect_dma_start(
                        out=ch[:, j, 0:D],
                        out_offset=None,
                        in_=table_t[:],
                        in_offset=bass.IndirectOffsetOnAxis(
                            ap=ids_sb[:, t : t + 1], axis=0
                        ),
                    )

                # sin(freq * tok) for the whole chunk: [128, 8 tiles, 100]
                tok_b = tokf[:, g * CHUNK_TILES : (g + 1) * CHUNK_TILES]
                x = wpool.tile([P, W], _f32, tag="x")
                nc.vector.tensor_tensor(
                    out=x[:].rearrange("p (j h) -> p j h", j=CHUNK_TILES),
                    in0=tok_b.to_broadcast([P, CHUNK_TILES, H]),
                    in1=freqs_sb[:].rearrange("p (j h) -> p j h", j=CHUNK_TILES),
                    op=mybir.AluOpType.mult,
                )
                kint = wpool.tile([P, W], _i32, tag="kint")
                nc.vector.tensor_scalar(
                    out=kint[:], in0=x[:], scalar1=INV2PI, scalar2=None,
                    op0=mybir.AluOpType.mult,
                )
                kf = wpool.tile([P, W], _f32, tag="kf")
                nc.vector.tensor_copy(out=kf[:], in_=kint[:])  # exact int->f32
                r = wpool.tile([P, W], _f32, tag="r")
                nc.vector.cody_waite_cascade(out=r[:], x=x[:], k=kf[:], c1=C1, c2=C2, c3=C3)
                r2 = wpool.tile([P, W], _f32, tag="r2")
                nc.vector.add_range_wrap(
                    out=r2[:], in_=r[:], shift=0.0, bound=PI_F32, period=TWO_PI_F32
                )
                r3 = wpool.tile([P, W], _f32, tag="r3")
                nc.vector.tensor_scalar(
                    out=r3[:], in0=r2[:], scalar1=PI_SAFE, scalar2=-PI_SAFE,
                    op0=mybir.AluOpType.min, op1=mybir.AluOpType.max,
                )
                nc.scalar.activation(
                    out=ch[:, :, D:OUT_D],
                    in_=r3[:].rearrange("p (j h) -> p j h", j=CHUNK_TILES),
                    func=mybir.ActivationFunctionType.Sin,
                )

                # store: token g*1024 + j*128 + p lives at ch[p, j, :]
                nc.sync.dma_start(
                    out=out_t[g * CHUNK_TILES * P : (g + 1) * CHUNK_TILES * P, :]
                    .rearrange("(j p) c -> p j c", p=P),
                    in_=ch[:],
                )
    nc.compile()
    return nc


def _get_nc():
    global _NC
    if _NC is None:
        _NC = _build_nc()
    return _NC


def kernel(word_ids, word_emb_table):
    ids = np.ascontiguousarray(np.asarray(word_ids)).astype(np.int32).reshape(-1)
    table = np.ascontiguousarray(np.asarray(word_emb_table, dtype=np.float32))
    freqs = np.arange(H, dtype=np.float32) / np.float32(1000.0)
    freqs_in = np.ascontiguousarray(
        np.broadcast_to(np.tile(freqs, CHUNK_TILES), (P, W))
    )

    nc = _get_nc()
    in_maps = []
    for c in range(N_CORES):
        shard = ids[c * T_CORE : (c + 1) * T_CORE]
        in_maps.append(
            {
                "ids": np.ascontiguousarray(shard.reshape(N_TILES, P).T),
                "freqs": freqs_in,
                "table": table,
            }
        )
    res = run_bass_kernel_spmd(nc, in_maps, core_ids=list(range(N_CORES)))
    out = np.concatenate([r["out"] for r in res.results], axis=0)
    return out.reshape(B, S, OUT_D)
